# revision 3
# baseline (speedup 1.0000x reference)
"""Trainium2 Bass kernel for nn_Attention2 (8-head encoder/decoder attention mix).

Reference computation (per full batch B=4096):
    enc_h  = relu(encoder_input @ W_enc + b_enc)               [B, 1024]
    heads  = relu(einsum('bh,khd->kbd', enc_h, W_heads) + b_heads)  [8, B, 1024]
    dec_H  = relu(decoder_input @ W_dec + b_dec)               [B, 1024]
    scores = sum(heads * dec_H, axis=2)                        [8, B]
    attn   = softmax(scores.T, axis=1)                         [B, 8]
    out    = einsum('kbd,bk->bd', heads, attn)                 [B, 1024]

Sharding: pure data-parallel over the batch dim across 8 NeuronCores
(B_loc = 512 per core, all params replicated, zero collectives).

Per-core plan (v2 — tuned from the 176 us baseline NTFF profile):
  - PE stream is at the bf16 roofline (216 ns per [128k,128m,512n] matmul);
    the wins are at the edges: startup DMA wait, HAM cold-start, mid-stream
    DMA-starvation gaps, and the serial tail after the last matmul.
  - Warm-up: a few dummy matmuls on constant tiles run during the initial
    DMA wait so the PE HAM un-throttles (1.2->2.4 GHz) before real work.
  - Startup DMAs spread across all three DGE queues (sync + scalar HWDGE,
    gpsimd SWDGE) and ordered so the first stage-A wave's strips land first
    (w_enc split into wave halves).
  - w_heads strips alternate sync/scalar queues (2 MB/head; a single queue
    at ~130 GB/s was barely keeping ahead of the 15.5 us/head PE stream).
  - Stage A (feature-major): enc_hT[hid, b] = relu(W_enc.T @ x_encT + b_enc);
    bias+relu fused on ScalarE (per-partition bias).
  - Stage C (batch-major): dec_bm[b, hid]; bias injected via K=128
    ones-matmul into PSUM (cheaper than any DVE alternative: DVE
    tensor_tensor on a [128,512] f32 PSUM source costs ~660 ns vs 216 ns
    of PE, and DVE is the second-busiest engine).
  - Stage B (batch-major, per head): same ones-matmul bias trick.
  - Scores via fused scalar_tensor_tensor (mult + free-dim accumulate),
    streaming normalizer-free softmax: e = exp(score - 24), out_acc
    accumulated per head (h=0 writes via tensor_scalar: no memset needed).
  - Finalization (reduce/recip/scale/store) inlined into the last head's
    b-loop so b0..b2 finish during b1..b3's matmuls; the last b-tile runs
    in 4x256-column chunks so the post-matmul dependency chain
    (relu->score->exp->accumulate->scale->DMA) is short; output DMA split
    across both HWDGE queues.

Measured (core 0, NTFF profile): 176.2 us baseline -> see test runs.
rel err ~4.0e-3 (bf16 matmuls, f32 accumulate/softmax).
"""

import os
import numpy as np
from contextlib import ExitStack

N_CORES = 8
ENC_DIM, DEC_DIM, HID, HEADS, BATCH = 1024, 512, 1024, 8, 4096
B_LOC = BATCH // N_CORES          # 512 batch rows per core
P = 128                           # SBUF partitions
NCHUNK = 512                      # matmul moving free-dim (1 PSUM bank f32)
SCORE_SHIFT = 24.0                # scores measured in [14.2, 34.0]

# matmul input dtype: "bf16" (1 cyc/row PE, rel err ~4e-3) or "f32r"
# (fp32 bits, ~2 cyc/row PE, rel err ~2.5e-4)
MM_DTYPE = os.environ.get("BASS_MM_DTYPE", "bf16")

_cache = {}


def _build(mm_dtype: str):
    import concourse.tile as tile
    from concourse import bacc, mybir

    f32 = mybir.dt.float32
    bf16 = mybir.dt.bfloat16
    MM = mybir.dt.float32r if mm_dtype == "f32r" else bf16
    ST = f32   # head/dec storage dtype
    Relu = mybir.ActivationFunctionType.Relu
    Exp = mybir.ActivationFunctionType.Exp
    X = mybir.AxisListType.X
    mult = mybir.AluOpType.mult

    N_WARM = int(os.environ.get("BASS_WARMUP", "5"))
    TAIL_CHUNK = int(os.environ.get("BASS_TAIL_CHUNK", "256"))

    KT_E = ENC_DIM // P           # 8 contraction tiles (enc dim)
    KT_H = HID // P               # 8 contraction tiles (hid dim)
    KT_D = DEC_DIM // P           # 4 contraction tiles (dec dim)
    MT = HID // P                 # 8 hid tiles (feature-major partitions)
    BT = B_LOC // P               # 4 batch tiles
    NC_H = HID // NCHUNK          # 2 moving chunks over hid

    nc = bacc.Bacc("TRN2", target_bir_lowering=False, debug=False,
                   num_devices=N_CORES)

    xeT = nc.dram_tensor("x_enc_t", [ENC_DIM, B_LOC], MM, kind="ExternalInput").ap()
    xdT = nc.dram_tensor("x_dec_t", [DEC_DIM, B_LOC], MM, kind="ExternalInput").ap()
    w_enc = nc.dram_tensor("w_enc", [ENC_DIM, HID], MM, kind="ExternalInput").ap()
    b_enc_pp = nc.dram_tensor("b_enc_pp", [P, MT], f32, kind="ExternalInput").ap()
    w_heads = nc.dram_tensor("w_heads", [HEADS, HID, HID], MM, kind="ExternalInput").ap()
    b_heads = nc.dram_tensor("b_heads_pad", [HEADS, P, HID], MM, kind="ExternalInput").ap()
    w_dec = nc.dram_tensor("w_dec", [DEC_DIM, HID], MM, kind="ExternalInput").ap()
    b_dec = nc.dram_tensor("b_dec_pad", [P, HID], MM, kind="ExternalInput").ap()
    out_d = nc.dram_tensor("out", [B_LOC, HID], f32, kind="ExternalOutput").ap()

    with tile.TileContext(nc) as tc, ExitStack() as ctx:
        persist = ctx.enter_context(tc.tile_pool(name="persist", bufs=1))
        psums = ctx.enter_context(tc.tile_pool(name="psums", bufs=8, space="PSUM"))

        # --- constants / biases ---
        ones1 = persist.tile([P, P], MM, tag="ones1", name="ones1")
        dumr = persist.tile([P, NCHUNK], MM, tag="dumr", name="dumr")
        if mm_dtype == "f32r":
            nc.vector.memset(ones1[:].bitcast(f32), 1.0)
            nc.vector.memset(dumr[:].bitcast(f32), 0.0)
        else:
            nc.vector.memset(ones1[:], 1.0)
            nc.vector.memset(dumr[:], 0.0)
        benc = persist.tile([P, MT], f32, tag="benc", name="benc")
        bhp = [persist.tile([P, HID], MM, tag=f"bhp{h}", name=f"bhp{h}")
               for h in range(HEADS)]
        bdp = persist.tile([P, HID], MM, tag="bdp", name="bdp")
        negC = persist.tile([P, 1], f32, tag="negC", name="negC")
        nc.vector.memset(negC[:], -SCORE_SHIFT)

        # --- PE warm-up: dummy matmuls during the initial DMA wait so the
        # HAM clock gate opens (1.2 -> 2.4 GHz) before real matmuls. ---
        for i in range(N_WARM):
            w = psums.tile([P, NCHUNK], f32, tag="mm", name=f"warm{i}")
            nc.tensor.matmul(w[:], ones1[:], dumr[:], start=True, stop=True)

        # --- persistent activations ---
        ench = [persist.tile([P, B_LOC], MM, tag=f"ench{m}", name=f"ench{m}") for m in range(MT)]
        dec_bm = [persist.tile([P, HID], ST, tag=f"dec{b}", name=f"dec{b}") for b in range(BT)]
        e_all = [persist.tile([P, HEADS], f32, tag=f"eall{b}", name=f"eall{b}") for b in range(BT)]
        out_acc = [persist.tile([P, HID], f32, tag=f"oacc{b}", name=f"oacc{b}") for b in range(BT)]

        # ---- Stage A (enc trunk, feature-major), k-outer in 2 waves of 4
        # m-tiles; w_enc strips are split into wave halves so wave 0 only
        # waits for 1 MB per queue. Then Stage C.
        with ExitStack() as actx:
            a_pool = actx.enter_context(tc.tile_pool(name="stageA", bufs=1))
            xe = [a_pool.tile([P, B_LOC], MM, tag=f"xe{k}", name=f"xe{k}") for k in range(KT_E)]
            we = [[a_pool.tile([P, NCHUNK], MM, tag=f"we{k}_{w}", name=f"we{k}_{w}")
                   for w in range(2)] for k in range(KT_E)]
            # scalar HWDGE queue: x_enc strips (needed first), then benc
            for k in range(KT_E):
                nc.scalar.dma_start(xe[k][:], xeT[k * P:(k + 1) * P, :])
            nc.scalar.dma_start(benc[:], b_enc_pp[:])
            # sync HWDGE queue: w_enc wave-0 halves first, then wave-1
            for w in range(2):
                for k in range(KT_E):
                    nc.sync.dma_start(we[k][w][:],
                                      w_enc[k * P:(k + 1) * P,
                                            w * NCHUNK:(w + 1) * NCHUNK])
            # gpsimd SWDGE queue: everything stage C / bias related
            xd = [a_pool.tile([P, B_LOC], MM, tag=f"xd{k}", name=f"xd{k}") for k in range(KT_D)]
            wd = [a_pool.tile([P, HID], MM, tag=f"wd{k}", name=f"wd{k}") for k in range(KT_D)]
            nc.gpsimd.dma_start(bdp[:], b_dec[:])
            for k in range(KT_D):
                nc.gpsimd.dma_start(xd[k][:], xdT[k * P:(k + 1) * P, :])
                nc.gpsimd.dma_start(wd[k][:], w_dec[k * P:(k + 1) * P, :])
            for h in range(HEADS):
                nc.gpsimd.dma_start(bhp[h][:], b_heads[h])

            for wave in range(2):
                mset = range(wave * MT // 2, (wave + 1) * MT // 2)
                pss = {}
                for m in mset:
                    pss[m] = psums.tile([P, B_LOC], f32, tag="mm", name="ps")
                for k in range(KT_E):
                    for m in mset:
                        mm = m - wave * (MT // 2)
                        nc.tensor.matmul(pss[m][:],
                                         we[k][wave][:, mm * P:(mm + 1) * P],
                                         xe[k][:],
                                         start=(k == 0), stop=(k == KT_E - 1))
                for m in mset:
                    nc.scalar.activation(ench[m][:], pss[m][:], Relu,
                                         bias=benc[:, m:m + 1], scale=1.0)

            for b in range(BT):
                for n in range(NC_H):
                    ps = psums.tile([P, NCHUNK], f32, tag="mm", name="ps")
                    ncol = slice(n * NCHUNK, (n + 1) * NCHUNK)
                    nc.tensor.matmul(ps[:], ones1[:], bdp[:, ncol],
                                     start=True, stop=False)
                    for k in range(KT_D):
                        nc.tensor.matmul(ps[:], xd[k][:, b * P:(b + 1) * P],
                                         wd[k][:, ncol],
                                         start=False, stop=(k == KT_D - 1))
                    nc.scalar.activation(dec_bm[b][:, ncol], ps[:], Relu)

        # ---- Stage B + D + F: heads (batch-major), streaming softmax ----
        wh_pool = ctx.enter_context(tc.tile_pool(name="wh", bufs=24))
        head_pool = ctx.enter_context(tc.tile_pool(name="head", bufs=3))
        scratch = ctx.enter_context(tc.tile_pool(name="scratch", bufs=4))
        fin = ctx.enter_context(tc.tile_pool(name="fin", bufs=2))

        for h in range(HEADS):
            wh = []
            for k in range(KT_H):
                t = wh_pool.tile([P, HID], MM, tag="whs", name="whs")
                eng = nc.sync if k % 2 == 0 else nc.scalar
                eng.dma_start(t[:], w_heads[h, k * P:(k + 1) * P, :])
                wh.append(t)
            for b in range(BT):
                last = (h == HEADS - 1 and b == BT - 1)
                chunks = ([TAIL_CHUNK] * (HID // TAIL_CHUNK) if last
                          else [NCHUNK] * NC_H)
                head_t = head_pool.tile([P, HID], ST, tag=f"head{b}", name=f"head{b}")
                prod = scratch.tile([P, HID], ST, tag="prod", name="prod")
                s_col = scratch.tile([P, 1], f32, tag="scol", name="scol")
                s_parts = []
                col = 0
                for ci, cw in enumerate(chunks):
                    ps = psums.tile([P, cw], f32, tag="mm", name="ps")
                    ncol = slice(col, col + cw)
                    nc.tensor.matmul(ps[:], ones1[:], bhp[h][:, ncol],
                                     start=True, stop=False)
                    for k in range(KT_H):
                        nc.tensor.matmul(ps[:], ench[k][:, b * P:(b + 1) * P],
                                         wh[k][:, ncol],
                                         start=False, stop=(k == KT_H - 1))
                    nc.scalar.activation(head_t[:, ncol], ps[:], Relu)
                    if last:
                        # per-chunk partial scores so the tail chain is short
                        sp = scratch.tile([P, 1], f32, tag=f"sp{ci}", name=f"sp{ci}")
                        nc.vector.scalar_tensor_tensor(
                            prod[:, ncol], head_t[:, ncol], 1.0,
                            dec_bm[b][:, ncol],
                            op0=mult, op1=mult, accum_out=sp[:])
                        s_parts.append(sp)
                        if ci % 2 == 1:
                            # pairwise tree add as partials become ready
                            dst = s_parts[ci - 1]
                            nc.vector.tensor_add(dst[:], dst[:], sp[:])
                    col += cw
                if last:
                    pair_sums = [s_parts[i] for i in range(0, len(s_parts), 2)]
                    acc = pair_sums[0]
                    for q in pair_sums[1:]:
                        nc.vector.tensor_add(acc[:], acc[:], q[:])
                    s_col = acc
                else:
                    nc.vector.scalar_tensor_tensor(
                        prod[:], head_t[:], 1.0, dec_bm[b][:],
                        op0=mult, op1=mult, accum_out=s_col[:])
                # e = exp(score - C)
                nc.scalar.activation(e_all[b][:, h:h + 1], s_col[:], Exp,
                                     bias=negC[:], scale=1.0)
                # fin part 1 (emit before the accumulate ops so the DVE can
                # interleave: reduce/recip only need e_all)
                if h == HEADS - 1:
                    s_sum = fin.tile([P, 1], f32, tag="ssum", name="ssum")
                    rinv = fin.tile([P, 1], f32, tag="rinv", name="rinv")
                    nc.vector.reduce_sum(s_sum[:], e_all[b][:], axis=X)
                    nc.vector.reciprocal(rinv[:], s_sum[:])
                # out_acc update (h=0 writes, so no memset needed; h=7 is
                # handled in fin part 2 below, fused with scale+store)
                e_ap = e_all[b][:, h:h + 1]
                if h == 0:
                    nc.vector.tensor_scalar_mul(out_acc[b][:], head_t[:], e_ap)
                elif h < HEADS - 1:
                    nc.vector.scalar_tensor_tensor(
                        out_acc[b][:], head_t[:], e_ap,
                        out_acc[b][:], op0=mult, op1=mybir.AluOpType.add)
                # fin part 2: scale + store
                if h == HEADS - 1:
                    out_f = fin.tile([P, HID], f32, tag="outf", name="outf")
                    if not last:
                        nc.vector.scalar_tensor_tensor(
                            out_acc[b][:], head_t[:], e_ap,
                            out_acc[b][:], op0=mult, op1=mybir.AluOpType.add)
                        nc.vector.tensor_scalar_mul(out_f[:], out_acc[b][:], rinv[:])
                        eng = nc.sync if b % 2 == 0 else nc.scalar
                        eng.dma_start(out_d[b * P:(b + 1) * P, :], out_f[:])
                    else:
                        # chunked: accumulate+scale+store per chunk so the
                        # output DMAs start as soon as each chunk is ready
                        col = 0
                        for ci, cw in enumerate(chunks):
                            ncol = slice(col, col + cw)
                            nc.vector.scalar_tensor_tensor(
                                out_acc[b][:, ncol], head_t[:, ncol], e_ap,
                                out_acc[b][:, ncol],
                                op0=mult, op1=mybir.AluOpType.add)
                            nc.vector.tensor_scalar_mul(
                                out_f[:, ncol], out_acc[b][:, ncol], rinv[:])
                            eng = nc.sync if ci % 2 == 0 else nc.scalar
                            eng.dma_start(out_d[b * P:(b + 1) * P, ncol],
                                          out_f[:, ncol])
                            col += cw

    nc.compile()
    return nc


def _get_nc():
    if MM_DTYPE not in _cache:
        _cache[MM_DTYPE] = _build(MM_DTYPE)
    return _cache[MM_DTYPE]


def build_in_maps(encoder_input, decoder_input, W_enc, b_enc, W_heads,
                  b_heads, W_dec, b_dec):
    if MM_DTYPE == "bf16":
        import ml_dtypes
        cast = lambda a: np.ascontiguousarray(np.asarray(a, dtype=np.float32)).astype(ml_dtypes.bfloat16)
    else:
        cast = lambda a: np.ascontiguousarray(np.asarray(a, dtype=np.float32))

    xeT = cast(np.asarray(encoder_input).T)            # [1024, 4096]
    xdT = cast(np.asarray(decoder_input).T)            # [512, 4096]
    bh_pad = np.zeros((HEADS, P, HID), np.float32)
    bh_pad[:, 0, :] = np.asarray(b_heads, dtype=np.float32)
    bd_pad = np.zeros((P, HID), np.float32)
    bd_pad[0, :] = np.asarray(b_dec, dtype=np.float32)
    shared = {
        "w_enc": cast(W_enc),
        "b_enc_pp": np.ascontiguousarray(
            np.asarray(b_enc, dtype=np.float32).reshape(HID // P, P).T),
        "w_heads": cast(W_heads),
        "b_heads_pad": cast(bh_pad),
        "w_dec": cast(W_dec),
        "b_dec_pad": cast(bd_pad),
    }
    in_maps = []
    for c in range(N_CORES):
        sl = slice(c * B_LOC, (c + 1) * B_LOC)
        m = dict(shared)
        m["x_enc_t"] = np.ascontiguousarray(xeT[:, sl])
        m["x_dec_t"] = np.ascontiguousarray(xdT[:, sl])
        in_maps.append(m)
    return in_maps


def kernel(encoder_input, decoder_input, W_enc, b_enc, W_heads, b_heads,
           W_dec, b_dec):
    from concourse.bass_utils import run_bass_kernel_spmd

    nc = _get_nc()
    in_maps = build_in_maps(encoder_input, decoder_input, W_enc, b_enc,
                            W_heads, b_heads, W_dec, b_dec)
    res = run_bass_kernel_spmd(nc, in_maps, list(range(N_CORES)))
    out = np.concatenate([res.results[c]["out"] for c in range(N_CORES)], axis=0)
    return out.astype(np.float32)


# revision 6
# speedup vs baseline: 1.0431x; 1.0431x over previous
"""Trainium2 Bass kernel for nn_Attention2 (8-head encoder/decoder attention mix).

Reference computation (per full batch B=4096):
    enc_h  = relu(encoder_input @ W_enc + b_enc)               [B, 1024]
    heads  = relu(einsum('bh,khd->kbd', enc_h, W_heads) + b_heads)  [8, B, 1024]
    dec_H  = relu(decoder_input @ W_dec + b_dec)               [B, 1024]
    scores = sum(heads * dec_H, axis=2)                        [8, B]
    attn   = softmax(scores.T, axis=1)                         [B, 8]
    out    = einsum('kbd,bk->bd', heads, attn)                 [B, 1024]

Sharding: pure data-parallel over the batch dim across 8 NeuronCores
(B_loc = 512 per core, all params replicated, zero collectives).

Per-core plan (v2 — tuned from the 176 us baseline NTFF profile):
  - PE stream is at the bf16 roofline (216 ns per [128k,128m,512n] matmul);
    the wins are at the edges: startup DMA wait, HAM cold-start, mid-stream
    DMA-starvation gaps, and the serial tail after the last matmul.
  - Warm-up: a few dummy matmuls on constant tiles run during the initial
    DMA wait so the PE HAM un-throttles (1.2->2.4 GHz) before real work.
  - Startup DMAs spread across all three DGE queues (sync + scalar HWDGE,
    gpsimd SWDGE) and ordered so the first stage-A wave's strips land first
    (w_enc split into wave halves).
  - w_heads strips alternate sync/scalar queues (2 MB/head; a single queue
    at ~130 GB/s was barely keeping ahead of the 15.5 us/head PE stream).
  - Stage A (feature-major): enc_hT[hid, b] = relu(W_enc.T @ x_encT + b_enc);
    bias+relu fused on ScalarE (per-partition bias).
  - Stage C (batch-major): dec_bm[b, hid]; bias injected via K=128
    ones-matmul into PSUM (cheaper than any DVE alternative: DVE
    tensor_tensor on a [128,512] f32 PSUM source costs ~660 ns vs 216 ns
    of PE, and DVE is the second-busiest engine).
  - Stage B (batch-major, per head): same ones-matmul bias trick.
  - Scores via fused scalar_tensor_tensor (mult + free-dim accumulate),
    streaming normalizer-free softmax: e = exp(score - 24), out_acc
    accumulated per head (h=0 writes via tensor_scalar: no memset needed).
  - Finalization (reduce/recip/scale/store) inlined into the last head's
    b-loop so b0..b2 finish during b1..b3's matmuls; the last b-tile runs
    in 4x256-column chunks so the post-matmul dependency chain
    (relu->score->exp->accumulate->scale->DMA) is short; output DMA split
    across both HWDGE queues.

Measured (core 0, NTFF profile): 176.2 us baseline -> see test runs.
rel err ~4.0e-3 (bf16 matmuls, f32 accumulate/softmax).
"""

import os
import numpy as np
from contextlib import ExitStack

N_CORES = 8
ENC_DIM, DEC_DIM, HID, HEADS, BATCH = 1024, 512, 1024, 8, 4096
B_LOC = BATCH // N_CORES          # 512 batch rows per core
P = 128                           # SBUF partitions
NCHUNK = 512                      # matmul moving free-dim (1 PSUM bank f32)
SCORE_SHIFT = 24.0                # scores measured in [14.2, 34.0]

# matmul input dtype: "bf16" (1 cyc/row PE, rel err ~4e-3) or "f32r"
# (fp32 bits, ~2 cyc/row PE, rel err ~2.5e-4)
MM_DTYPE = os.environ.get("BASS_MM_DTYPE", "bf16")

_cache = {}


def _build(mm_dtype: str):
    import concourse.tile as tile
    from concourse import bacc, mybir

    f32 = mybir.dt.float32
    bf16 = mybir.dt.bfloat16
    MM = mybir.dt.float32r if mm_dtype == "f32r" else bf16
    ST = f32   # head/dec storage dtype
    Relu = mybir.ActivationFunctionType.Relu
    Exp = mybir.ActivationFunctionType.Exp
    X = mybir.AxisListType.X
    mult = mybir.AluOpType.mult

    N_WARM = int(os.environ.get("BASS_WARMUP", "5"))
    TAIL_CHUNK = int(os.environ.get("BASS_TAIL_CHUNK", "256"))

    KT_E = ENC_DIM // P           # 8 contraction tiles (enc dim)
    KT_H = HID // P               # 8 contraction tiles (hid dim)
    KT_D = DEC_DIM // P           # 4 contraction tiles (dec dim)
    MT = HID // P                 # 8 hid tiles (feature-major partitions)
    BT = B_LOC // P               # 4 batch tiles
    NC_H = HID // NCHUNK          # 2 moving chunks over hid

    nc = bacc.Bacc("TRN2", target_bir_lowering=False, debug=False,
                   num_devices=N_CORES)

    xeT = nc.dram_tensor("x_enc_t", [ENC_DIM, B_LOC], MM, kind="ExternalInput").ap()
    xdT = nc.dram_tensor("x_dec_t", [DEC_DIM, B_LOC], MM, kind="ExternalInput").ap()
    w_enc = nc.dram_tensor("w_enc", [ENC_DIM, HID], MM, kind="ExternalInput").ap()
    b_enc_pp = nc.dram_tensor("b_enc_pp", [P, MT], f32, kind="ExternalInput").ap()
    w_heads = nc.dram_tensor("w_heads", [HEADS, HID, HID], MM, kind="ExternalInput").ap()
    b_heads = nc.dram_tensor("b_heads_pad", [HEADS, P, HID], MM, kind="ExternalInput").ap()
    w_dec = nc.dram_tensor("w_dec", [DEC_DIM, HID], MM, kind="ExternalInput").ap()
    b_dec = nc.dram_tensor("b_dec_pad", [P, HID], MM, kind="ExternalInput").ap()
    out_d = nc.dram_tensor("out", [B_LOC, HID], f32, kind="ExternalOutput").ap()

    with tile.TileContext(nc) as tc, ExitStack() as ctx:
        persist = ctx.enter_context(tc.tile_pool(name="persist", bufs=1))
        psums = ctx.enter_context(tc.tile_pool(name="psums", bufs=8, space="PSUM"))

        # --- constants / biases ---
        ones1 = persist.tile([P, P], MM, tag="ones1", name="ones1")
        dumr = persist.tile([P, NCHUNK], MM, tag="dumr", name="dumr")
        if mm_dtype == "f32r":
            nc.vector.memset(ones1[:].bitcast(f32), 1.0)
            nc.vector.memset(dumr[:].bitcast(f32), 0.0)
        else:
            nc.vector.memset(ones1[:], 1.0)
            nc.vector.memset(dumr[:], 0.0)
        benc = persist.tile([P, MT], f32, tag="benc", name="benc")
        bhp = [persist.tile([P, HID], MM, tag=f"bhp{h}", name=f"bhp{h}")
               for h in range(HEADS)]
        bdp = persist.tile([P, HID], MM, tag="bdp", name="bdp")
        negC = persist.tile([P, 1], f32, tag="negC", name="negC")
        nc.vector.memset(negC[:], -SCORE_SHIFT)

        # --- PE warm-up: dummy matmuls during the initial DMA wait so the
        # HAM clock gate opens (1.2 -> 2.4 GHz) before real matmuls. ---
        for i in range(N_WARM):
            w = psums.tile([P, NCHUNK], f32, tag="mm", name=f"warm{i}")
            nc.tensor.matmul(w[:], ones1[:], dumr[:], start=True, stop=True)

        # --- persistent activations ---
        ench = [persist.tile([P, B_LOC], MM, tag=f"ench{m}", name=f"ench{m}") for m in range(MT)]
        dec_bm = [persist.tile([P, HID], ST, tag=f"dec{b}", name=f"dec{b}") for b in range(BT)]
        e_all = [persist.tile([P, HEADS], f32, tag=f"eall{b}", name=f"eall{b}") for b in range(BT)]
        out_acc = [persist.tile([P, HID], f32, tag=f"oacc{b}", name=f"oacc{b}") for b in range(BT)]

        # ---- Stage A (enc trunk, feature-major), k-outer in 2 waves of 4
        # m-tiles; w_enc strips are split into wave halves so wave 0 only
        # waits for 1 MB per queue. Then Stage C.
        with ExitStack() as actx:
            a_pool = actx.enter_context(tc.tile_pool(name="stageA", bufs=1))
            xe = [a_pool.tile([P, B_LOC], MM, tag=f"xe{k}", name=f"xe{k}") for k in range(KT_E)]
            we = [[a_pool.tile([P, NCHUNK], MM, tag=f"we{k}_{w}", name=f"we{k}_{w}")
                   for w in range(2)] for k in range(KT_E)]
            # Queue discipline: ONLY the two HWDGE queues carry input
            # streams, strictly in need-order, so the critical stage-A
            # strips are never bandwidth-starved (SDMA engines round-robin
            # across queues with pending work, so a third busy queue would
            # steal ~1/3 of HBM bandwidth exactly when xe/we are critical).
            # scalar HWDGE queue: x_enc strips, benc, then stage-C strips
            for k in range(KT_E):
                nc.scalar.dma_start(xe[k][:], xeT[k * P:(k + 1) * P, :])
            nc.scalar.dma_start(benc[:], b_enc_pp[:])
            xd = [a_pool.tile([P, B_LOC], MM, tag=f"xd{k}", name=f"xd{k}") for k in range(KT_D)]
            wd = [a_pool.tile([P, HID], MM, tag=f"wd{k}", name=f"wd{k}") for k in range(KT_D)]
            for k in range(KT_D):
                nc.scalar.dma_start(xd[k][:], xdT[k * P:(k + 1) * P, :])
                nc.scalar.dma_start(wd[k][:], w_dec[k * P:(k + 1) * P, :])
            # sync HWDGE queue: w_enc wave-0 halves, wave-1 halves, bdp
            # (bhp[h] is emitted inside the head loop, one head ahead)
            for w in range(2):
                for k in range(KT_E):
                    nc.sync.dma_start(we[k][w][:],
                                      w_enc[k * P:(k + 1) * P,
                                            w * NCHUNK:(w + 1) * NCHUNK])
            nc.sync.dma_start(bdp[:], b_dec[:])

            for wave in range(2):
                mset = range(wave * MT // 2, (wave + 1) * MT // 2)
                pss = {}
                for m in mset:
                    pss[m] = psums.tile([P, B_LOC], f32, tag="mm", name="ps")
                for k in range(KT_E):
                    for m in mset:
                        mm = m - wave * (MT // 2)
                        nc.tensor.matmul(pss[m][:],
                                         we[k][wave][:, mm * P:(mm + 1) * P],
                                         xe[k][:],
                                         start=(k == 0), stop=(k == KT_E - 1))
                for m in mset:
                    nc.scalar.activation(ench[m][:], pss[m][:], Relu,
                                         bias=benc[:, m:m + 1], scale=1.0)

            for b in range(BT):
                for n in range(NC_H):
                    ps = psums.tile([P, NCHUNK], f32, tag="mm", name="ps")
                    ncol = slice(n * NCHUNK, (n + 1) * NCHUNK)
                    nc.tensor.matmul(ps[:], ones1[:], bdp[:, ncol],
                                     start=True, stop=False)
                    for k in range(KT_D):
                        nc.tensor.matmul(ps[:], xd[k][:, b * P:(b + 1) * P],
                                         wd[k][:, ncol],
                                         start=False, stop=(k == KT_D - 1))
                    nc.scalar.activation(dec_bm[b][:, ncol], ps[:], Relu)

        # ---- Stage B + D + F: heads (batch-major), streaming softmax ----
        wh_pool = ctx.enter_context(tc.tile_pool(name="wh", bufs=24))
        head_pool = ctx.enter_context(tc.tile_pool(name="head", bufs=3))
        scratch = ctx.enter_context(tc.tile_pool(name="scratch", bufs=4))
        fin = ctx.enter_context(tc.tile_pool(name="fin", bufs=2))

        for h in range(HEADS):
            nc.sync.dma_start(bhp[h][:], b_heads[h])
            wh = []
            for k in range(KT_H):
                t = wh_pool.tile([P, HID], MM, tag="whs", name="whs")
                eng = nc.sync if k % 2 == 0 else nc.scalar
                eng.dma_start(t[:], w_heads[h, k * P:(k + 1) * P, :])
                wh.append(t)
            for b in range(BT):
                last = (h == HEADS - 1 and b == BT - 1)
                chunks = ([TAIL_CHUNK] * (HID // TAIL_CHUNK) if last
                          else [NCHUNK] * NC_H)
                head_t = head_pool.tile([P, HID], ST, tag=f"head{b}", name=f"head{b}")
                prod = scratch.tile([P, HID], ST, tag="prod", name="prod")
                s_col = scratch.tile([P, 1], f32, tag="scol", name="scol")
                s_parts = []
                col = 0
                for ci, cw in enumerate(chunks):
                    ps = psums.tile([P, cw], f32, tag="mm", name="ps")
                    ncol = slice(col, col + cw)
                    nc.tensor.matmul(ps[:], ones1[:], bhp[h][:, ncol],
                                     start=True, stop=False)
                    for k in range(KT_H):
                        nc.tensor.matmul(ps[:], ench[k][:, b * P:(b + 1) * P],
                                         wh[k][:, ncol],
                                         start=False, stop=(k == KT_H - 1))
                    nc.scalar.activation(head_t[:, ncol], ps[:], Relu)
                    if last:
                        # per-chunk partial scores so the tail chain is short
                        sp = scratch.tile([P, 1], f32, tag=f"sp{ci}", name=f"sp{ci}")
                        nc.vector.scalar_tensor_tensor(
                            prod[:, ncol], head_t[:, ncol], 1.0,
                            dec_bm[b][:, ncol],
                            op0=mult, op1=mult, accum_out=sp[:])
                        s_parts.append(sp)
                        if ci % 2 == 1:
                            # pairwise tree add as partials become ready
                            dst = s_parts[ci - 1]
                            nc.vector.tensor_add(dst[:], dst[:], sp[:])
                    col += cw
                if last:
                    pair_sums = [s_parts[i] for i in range(0, len(s_parts), 2)]
                    acc = pair_sums[0]
                    for q in pair_sums[1:]:
                        nc.vector.tensor_add(acc[:], acc[:], q[:])
                    s_col = acc
                else:
                    nc.vector.scalar_tensor_tensor(
                        prod[:], head_t[:], 1.0, dec_bm[b][:],
                        op0=mult, op1=mult, accum_out=s_col[:])
                # e = exp(score - C)
                nc.scalar.activation(e_all[b][:, h:h + 1], s_col[:], Exp,
                                     bias=negC[:], scale=1.0)
                # fin part 1 (emit before the accumulate ops so the DVE can
                # interleave: reduce/recip only need e_all)
                if h == HEADS - 1:
                    s_sum = fin.tile([P, 1], f32, tag="ssum", name="ssum")
                    rinv = fin.tile([P, 1], f32, tag="rinv", name="rinv")
                    nc.vector.reduce_sum(s_sum[:], e_all[b][:], axis=X)
                    nc.vector.reciprocal(rinv[:], s_sum[:])
                # out_acc update (h=0 writes, so no memset needed; h=7 is
                # handled in fin part 2 below, fused with scale+store)
                e_ap = e_all[b][:, h:h + 1]
                if h == 0:
                    nc.vector.tensor_scalar_mul(out_acc[b][:], head_t[:], e_ap)
                elif h < HEADS - 1:
                    nc.vector.scalar_tensor_tensor(
                        out_acc[b][:], head_t[:], e_ap,
                        out_acc[b][:], op0=mult, op1=mybir.AluOpType.add)
                # fin part 2: scale + store. The final scale (out_acc*rinv)
                # runs on ScalarE (ACT Copy with per-partition scale AP) so
                # the tail-critical DVE FIFO only carries the accumulates.
                if h == HEADS - 1:
                    out_f = fin.tile([P, HID], f32, tag="outf", name="outf")
                    if not last:
                        nc.vector.scalar_tensor_tensor(
                            out_acc[b][:], head_t[:], e_ap,
                            out_acc[b][:], op0=mult, op1=mybir.AluOpType.add)
                        nc.scalar.mul(out_f[:], out_acc[b][:], rinv[:])
                        nc.gpsimd.dma_start(out_d[b * P:(b + 1) * P, :], out_f[:])
                    else:
                        # last tile: accumulate+scale+store in 512-col chunks
                        # so each output DMA starts as soon as its chunk is
                        # ready; DMAs split across both HWDGE queues
                        for ci in range(NC_H):
                            ncol = slice(ci * NCHUNK, (ci + 1) * NCHUNK)
                            nc.vector.scalar_tensor_tensor(
                                out_acc[b][:, ncol], head_t[:, ncol], e_ap,
                                out_acc[b][:, ncol],
                                op0=mult, op1=mybir.AluOpType.add)
                            nc.scalar.mul(out_f[:, ncol],
                                          out_acc[b][:, ncol], rinv[:])
                            eng = nc.sync if ci % 2 == 0 else nc.scalar
                            eng.dma_start(out_d[b * P:(b + 1) * P, ncol],
                                          out_f[:, ncol])

    nc.compile()
    return nc


def _get_nc():
    if MM_DTYPE not in _cache:
        _cache[MM_DTYPE] = _build(MM_DTYPE)
    return _cache[MM_DTYPE]


def build_in_maps(encoder_input, decoder_input, W_enc, b_enc, W_heads,
                  b_heads, W_dec, b_dec):
    if MM_DTYPE == "bf16":
        import ml_dtypes
        cast = lambda a: np.ascontiguousarray(np.asarray(a, dtype=np.float32)).astype(ml_dtypes.bfloat16)
    else:
        cast = lambda a: np.ascontiguousarray(np.asarray(a, dtype=np.float32))

    xeT = cast(np.asarray(encoder_input).T)            # [1024, 4096]
    xdT = cast(np.asarray(decoder_input).T)            # [512, 4096]
    bh_pad = np.zeros((HEADS, P, HID), np.float32)
    bh_pad[:, 0, :] = np.asarray(b_heads, dtype=np.float32)
    bd_pad = np.zeros((P, HID), np.float32)
    bd_pad[0, :] = np.asarray(b_dec, dtype=np.float32)
    shared = {
        "w_enc": cast(W_enc),
        "b_enc_pp": np.ascontiguousarray(
            np.asarray(b_enc, dtype=np.float32).reshape(HID // P, P).T),
        "w_heads": cast(W_heads),
        "b_heads_pad": cast(bh_pad),
        "w_dec": cast(W_dec),
        "b_dec_pad": cast(bd_pad),
    }
    in_maps = []
    for c in range(N_CORES):
        sl = slice(c * B_LOC, (c + 1) * B_LOC)
        m = dict(shared)
        m["x_enc_t"] = np.ascontiguousarray(xeT[:, sl])
        m["x_dec_t"] = np.ascontiguousarray(xdT[:, sl])
        in_maps.append(m)
    return in_maps


def kernel(encoder_input, decoder_input, W_enc, b_enc, W_heads, b_heads,
           W_dec, b_dec):
    from concourse.bass_utils import run_bass_kernel_spmd

    nc = _get_nc()
    in_maps = build_in_maps(encoder_input, decoder_input, W_enc, b_enc,
                            W_heads, b_heads, W_dec, b_dec)
    res = run_bass_kernel_spmd(nc, in_maps, list(range(N_CORES)))
    out = np.concatenate([res.results[c]["out"] for c in range(N_CORES)], axis=0)
    return out.astype(np.float32)
